# revision 8
# baseline (speedup 1.0000x reference)
"""Trainium2 Bass kernel for GPTQMarlinFP8Linear: C = A @ (W*s)^T + b.

Shapes: A [4, 2048, 4096] f32, W [4096, 4096] f32 (values exactly on the
fp8-e4m3 grid), scales [4096] f32, bias [4096] f32 -> C [4, 2048, 4096] f32.

Strategy:
  - W is exactly representable in fp8-e4m3 (it came from an fp8 checkpoint),
    so cast W -> fp8 losslessly and run the matmul in fp8 DoubleRow mode
    (2 contraction elements per PE per cycle; 2x the fp16 rate).
  - A -> fp8 alone costs ~2.7e-2 relative error (too close to the 2e-2 gate),
    so split A = A_hi(fp8) + A_lo and apply a residual fp8 correction matmul
    on a fraction KLO/KO of the contraction dim. The residual reuses the
    same W tiles and accumulates into the same PSUM group, so the only extra
    cost is KLO/KO more DoubleRow matmuls. KLO=20/32 -> ~1.6e-2 rel err.
  - PSUM accumulates in fp32; dequant scale and bias are applied in fp32 at
    PSUM eviction (per-out-channel == per-partition, single DVE op).
  - 8 cores: 2-way shard over out_features (O) x 4-way over tokens (M).
    Each core computes a C^T block [O_sh=2048, M_sh=2048] with W as the
    stationary operand (lhsT) so output partitions = out channels.
  - Host pre-packs A/W tiles so every DMA is partition-major contiguous,
    and transposes C^T -> C at the end.
"""

import ml_dtypes
import numpy as np

import concourse.bass as bass
import concourse.mybir as mybir
import concourse.tile as tile
from concourse import bacc
from concourse.bass_utils import run_bass_kernel_spmd

# Problem shape
B, S, IN, OUT = 4, 2048, 4096, 4096
M = B * S            # 8192 tokens
K = IN               # 4096 contraction
O = OUT              # 4096 out channels

# Sharding: GO-way over out channels, GM-way over tokens (GO*GM == 8 cores)
GO, GM = 2, 4
O_SH = O // GO       # 2048
M_SH = M // GM       # 2048

P = 128              # partitions
KO = K // P          # 32 k-subtiles
KLO = 16             # k-subtiles receiving the fp8 residual correction
MFREE = 512          # moving free dim per matmul (one PSUM bank of fp32)
OT = O_SH // P       # 16 o-tiles per core
MT = M_SH // MFREE   # 4 m-tiles per core

F8 = mybir.dt.float8e4
F32 = mybir.dt.float32
NP_F8 = ml_dtypes.float8_e4m3  # TRN FP8_EXP4 grid (max 240); our data << 240

_cache = {}


def _build_nc():
    """Build the SPMD program (identical on all 8 cores; data differs)."""
    nc = bacc.Bacc(None, target_bir_lowering=False)

    # Pre-packed inputs (host layout, partition-major contiguous tiles):
    #   a:  [MT, P, KO, MFREE]  fp8 -- a[mt, p, ko, mi] = fp8(A_sh[mt*512+mi, ko*128+p])
    #   al: [MT, P, KLO, MFREE] fp8 -- residual fp8(A - fp8(A)) for k-subtiles < KLO
    #   w:  [OT, P, KO, P]      fp8 -- w[ot, p, ko, oi] = W_sh[ot*128+oi, ko*128+p]
    #   sc/bs: [P, OT] f32 -- sc[p, ot] = scales_sh[ot*128+p]
    a_dram = nc.dram_tensor("a", [MT, P, KO, MFREE], F8, kind="ExternalInput")
    al_dram = nc.dram_tensor("al", [MT, P, KLO, MFREE], F8, kind="ExternalInput")
    w_dram = nc.dram_tensor("w", [OT, P, KO, P], F8, kind="ExternalInput")
    sc_dram = nc.dram_tensor("sc", [P, OT], F32, kind="ExternalInput")
    bs_dram = nc.dram_tensor("bs", [P, OT], F32, kind="ExternalInput")
    out_dram = nc.dram_tensor("out", [O_SH, M_SH], F32, kind="ExternalOutput")

    with tile.TileContext(nc) as tc:
        with (
            tc.tile_pool(name="apool", bufs=1) as apool,
            tc.tile_pool(name="wpool", bufs=1) as wpool,
            tc.tile_pool(name="cpool", bufs=1) as cpool,
            tc.tile_pool(name="opool", bufs=6) as opool,
            tc.tile_pool(name="psum", bufs=6, space="PSUM") as psum,
        ):
            # A shard and all 16 W tiles stay SBUF-resident for the whole
            # kernel (~160 KB/part). Queue assignment keeps the early HBM
            # bandwidth for what the first 90 us of compute actually needs:
            #   gpsimd: a0 only (chunked per k-pair so matmuls start early)
            #   scalar: al0 (chunked), then the out tiles
            #   sync:   w0 (4 chunks), w1..w15, sc/bs, then the deferred bulk
            #           a1/al1/a2/al2/a3/al3 (first needed at ~95/180/260 us)
            # Serializing the bulk A tiles behind W on one queue stops them
            # from starving the W stream while mt=0 is computing.
            a_tiles = []
            al_tiles = []
            w_tiles = []
            for mt in range(MT):
                a_tiles.append(
                    apool.tile([P, KO, MFREE], F8, name=f"a{mt}", tag=f"a{mt}")
                )
                al_tiles.append(
                    apool.tile([P, KLO, MFREE], F8, name=f"al{mt}", tag=f"al{mt}")
                )
            # a0/al0 chunks alternate between the gpsimd and scalar queues in
            # consumption order, so the first group's delivery rate doubles.
            for kp in range(KO // 2):
                eng = nc.gpsimd if kp % 2 == 0 else nc.scalar
                eng.dma_start(
                    a_tiles[0][:, 2 * kp : 2 * kp + 2, :],
                    a_dram[0, :, 2 * kp : 2 * kp + 2, :],
                )
            for kp in range(KLO // 2):
                eng = nc.gpsimd if kp % 2 == 0 else nc.scalar
                eng.dma_start(
                    al_tiles[0][:, 2 * kp : 2 * kp + 2, :],
                    al_dram[0, :, 2 * kp : 2 * kp + 2, :],
                )

            for ot in range(OT):
                wt = wpool.tile([P, KO, P], F8, name=f"w{ot}", tag=f"w{ot}")
                if ot == 0:
                    for kq in range(4):
                        nc.sync.dma_start(
                            wt[:, 8 * kq : 8 * kq + 8, :],
                            w_dram[ot, :, 8 * kq : 8 * kq + 8, :],
                        )
                else:
                    nc.sync.dma_start(wt[:], w_dram[ot])
                w_tiles.append(wt)

            sc_sb = cpool.tile([P, OT], F32, name="sc_sb")
            bs_sb = cpool.tile([P, OT], F32, name="bs_sb")
            nc.sync.dma_start(sc_sb[:], sc_dram[:])
            nc.sync.dma_start(bs_sb[:], bs_dram[:])
            for mt in range(1, MT):
                nc.sync.dma_start(a_tiles[mt][:], a_dram[mt])
                nc.sync.dma_start(al_tiles[mt][:], al_dram[mt])

            # mt outer / ot inner: the first matmul only needs w0 plus the
            # leading slices of a0/al0, and each a-tile is reused for 16 ot
            # iterations (~90 us) while later tiles stream in behind it.
            for mt in range(MT):
                for ot in range(OT):
                    wt = w_tiles[ot]
                    ps = psum.tile([P, MFREE], F32, name=f"ps{ot}_{mt}", tag="ps")
                    # hi pass: full K in DoubleRow fp8 (2 k-subtiles / matmul)
                    for kp in range(KO // 2):
                        nc.tensor.matmul(
                            ps[:],
                            lhsT=wt[:, 2 * kp : 2 * kp + 2, :],
                            rhs=a_tiles[mt][:, 2 * kp : 2 * kp + 2, :],
                            start=(kp == 0),
                            stop=False,
                            perf_mode=mybir.MatmulPerfMode.DoubleRow,
                        )
                    # lo pass: residual correction on the first KLO k-subtiles
                    for kp in range(KLO // 2):
                        nc.tensor.matmul(
                            ps[:],
                            lhsT=wt[:, 2 * kp : 2 * kp + 2, :],
                            rhs=al_tiles[mt][:, 2 * kp : 2 * kp + 2, :],
                            start=False,
                            stop=(kp == KLO // 2 - 1),
                            perf_mode=mybir.MatmulPerfMode.DoubleRow,
                        )
                    osb = opool.tile([P, MFREE], F32, name=f"o{ot}_{mt}", tag="o")
                    # C^T = psum * scale[o] + bias[o]  (per-partition scalars)
                    nc.vector.tensor_scalar(
                        osb[:],
                        ps[:],
                        sc_sb[:, ot : ot + 1],
                        bs_sb[:, ot : ot + 1],
                        mybir.AluOpType.mult,
                        mybir.AluOpType.add,
                    )
                    nc.scalar.dma_start(
                        out_dram[ot * P : (ot + 1) * P, mt * MFREE : (mt + 1) * MFREE],
                        osb[:],
                    )

    nc.compile()
    return nc


def _get_nc():
    if "nc" not in _cache:
        _cache["nc"] = _build_nc()
    return _cache["nc"]


def _pack_a(blk_hi, blk_lo):
    """[M_SH, K] fp8 pair -> ([MT, P, KO, MFREE], [MT, P, KLO, MFREE])."""
    hi = blk_hi.reshape(MT, MFREE, KO, P).transpose(0, 3, 2, 1)
    lo = blk_lo.reshape(MT, MFREE, KO, P)[:, :, :KLO, :].transpose(0, 3, 2, 1)
    return np.ascontiguousarray(hi), np.ascontiguousarray(lo)


def _prepack(A, weight, scales, bias):
    """Shard + quantize + tile-pack inputs for each of the 8 cores."""
    A2 = np.ascontiguousarray(A, dtype=np.float32).reshape(M, K)
    W = np.ascontiguousarray(weight, dtype=np.float32)
    s = np.asarray(scales, dtype=np.float32)
    b = np.asarray(bias, dtype=np.float32)

    a_sh = []
    for mb in range(GM):
        blk = A2[mb * M_SH : (mb + 1) * M_SH]
        hi = blk.astype(NP_F8)
        lo = (blk - hi.astype(np.float32)).astype(NP_F8)
        a_sh.append(_pack_a(hi, lo))

    w_sh = []
    sc_sh = []
    bs_sh = []
    for ob in range(GO):
        wb = W[ob * O_SH : (ob + 1) * O_SH].astype(NP_F8)
        # [O_SH, K] -> [OT, P(oi), KO, P(p)] -> [OT, P(p), KO, P(oi)]
        wb = wb.reshape(OT, P, KO, P).transpose(0, 3, 2, 1)
        w_sh.append(np.ascontiguousarray(wb))
        sc_sh.append(np.ascontiguousarray(s[ob * O_SH : (ob + 1) * O_SH].reshape(OT, P).T))
        bs_sh.append(np.ascontiguousarray(b[ob * O_SH : (ob + 1) * O_SH].reshape(OT, P).T))

    in_maps = []
    for c in range(8):
        ob, mb = c // GM, c % GM
        in_maps.append(
            {
                "a": a_sh[mb][0],
                "al": a_sh[mb][1],
                "w": w_sh[ob],
                "sc": sc_sh[ob],
                "bs": bs_sh[ob],
            }
        )
    return in_maps


def _run(inputs, trace=False):
    nc = _get_nc()
    in_maps = _prepack(
        inputs["A"], inputs["weight"], inputs["scales"], inputs["bias"]
    )
    br = run_bass_kernel_spmd(nc, in_maps, core_ids=list(range(8)), trace=trace)

    CT = np.empty((O, M), dtype=np.float32)
    for c in range(8):
        ob, mb = c // GM, c % GM
        CT[ob * O_SH : (ob + 1) * O_SH, mb * M_SH : (mb + 1) * M_SH] = br.results[c][
            "out"
        ]
    C = np.ascontiguousarray(CT.T).reshape(B, S, O)
    return C, br


def kernel(**inputs) -> np.ndarray:
    return _run(inputs, trace=False)[0]


def kernel_traced(**inputs):
    """Like kernel() but with NTFF profiling; returns (C, BassKernelResults)."""
    return _run(inputs, trace=True)
